# revision 5
# baseline (speedup 1.0000x reference)
"""Cross-attention (B=2, N=2048, M=4096, H=16, dh=64) on 8 TRN2 NeuronCores.

Sharding: core c handles batch b=c//4 and head-group g=c%4 (4 heads, 256 of
the 1024 inner dims). Each core computes its partial out-projection
(x_b @ Wq_g ... @ Wo_g); the host sums the 4 partials per batch and adds bo.
No cross-core communication inside the kernel.

Per-core kernel layout (all matmuls in float32r = full PE rate):
  ctx   --PE transpose--> ctx^T --> k^T [d, m] and v [m, d] (+ones col)
  x     --PE transpose--> x^T   --> q^T [d, n]
  S^T[m-tile, n] = k_h^T.T @ q_h^T           (PSUM)
  P^T = exp(S^T * scale + logmask[m])         (ACT, PSUM->SBUF)
  acc[65, n] += [v_h | 1].T @ P^T             (PSUM accumulate over m-tiles;
                                               row 64 = softmax denominator)
  o^T_h = acc[0:64] * (1/acc[64]) broadcast   (DVE + broadcast DMA)
  out[n, :] = sum_pairs o^T.T @ Wo_g          (partial; host reduces)
"""

from contextlib import ExitStack
from functools import lru_cache

import numpy as np

import concourse.bass as bass
import concourse.mybir as mybir
import concourse.tile as tile
from concourse import bacc
from concourse.bass_utils import run_bass_kernel_spmd
from concourse.masks import make_identity

F32 = mybir.dt.float32
F32R = mybir.dt.float32r
AF = mybir.ActivationFunctionType

N_CORES = 8
B, N, M = 2, 2048, 4096
DQ, DC = 1024, 768          # query dim, context dim
H, DH = 16, 64              # total heads, head dim
HPC = 4                     # heads per core
GD = HPC * DH               # 256 inner dims per core
SCALE = DH ** -0.5
NEG = -30000.0              # additive mask value for masked-out positions

FQ = DQ // 128              # 8 feature tiles of x
FC = DC // 128              # 6 feature tiles of ctx
NT = N // 128               # 16 query tiles
MT = M // 128               # 32 context tiles
VW = DH + 1                 # 65: v columns + ones column


def build_kernel(tc: tile.TileContext, ins: dict, outs: dict):
    nc = tc.nc
    x_d, ctx_d = ins["x"], ins["ctx"]
    wq_d, wk_d, wv_d, wo_d, lm_d = (
        ins["wq"], ins["wk"], ins["wv"], ins["wo"], ins["lm"])
    out_d = outs["out"]

    es = ExitStack()
    with es:
        const = es.enter_context(tc.tile_pool(name="const", bufs=1))
        wpool = es.enter_context(tc.tile_pool(name="weights", bufs=1))
        persist = es.enter_context(tc.tile_pool(name="persist", bufs=1))

        ident = const.tile([128, 128], F32)
        make_identity(nc, ident)
        lm_sb = const.tile([128, MT], F32)
        nc.sync.dma_start(out=lm_sb, in_=lm_d)

        wq_sb = wpool.tile([128, FQ, GD], F32R)
        nc.sync.dma_start(out=wq_sb, in_=wq_d.rearrange("(t p) d -> p t d", p=128))
        wk_sb = wpool.tile([128, FC, GD], F32R)
        nc.sync.dma_start(out=wk_sb, in_=wk_d.rearrange("(t p) d -> p t d", p=128))
        wv_sb = wpool.tile([128, FC, GD], F32R)
        nc.sync.dma_start(out=wv_sb, in_=wv_d.rearrange("(t p) d -> p t d", p=128))
        wo_sb = wpool.tile([128, 2, DQ], F32R)
        nc.sync.dma_start(out=wo_sb, in_=wo_d.rearrange("(t p) d -> p t d", p=128))

        kT_sb = persist.tile([128, 2, M], F32R)    # [d within pair, pair, m]
        v_sb = persist.tile([128, MT, HPC * VW], F32R)  # [m within tile, mt, h*65+..]
        qT_sb = persist.tile([128, 2, N], F32R)
        oT_sb = persist.tile([128, 2, N], F32R)

        # ones columns of v (softmax denominator accumulators)
        ones32 = const.tile([128, 1], F32)
        nc.vector.memset(ones32, 1.0)
        for h in range(HPC):
            nc.vector.tensor_copy(
                out=v_sb[:, :, h * VW + DH : h * VW + DH + 1],
                in_=ones32.unsqueeze(1).to_broadcast([128, MT, 1]))

        # ---------------- Phase A: projections ----------------
        with (
            tc.tile_pool(name="ld", bufs=6) as ld_pool,
            tc.tile_pool(name="tT", bufs=2) as tT_pool,
            tc.tile_pool(name="tp_ps", bufs=2, space="PSUM") as tp_psum,
            tc.tile_pool(name="mm_ps", bufs=2, space="PSUM") as mm_psum,
            tc.tile_pool(name="v_ps", bufs=2, space="PSUM") as v_psum,
        ):
            # context -> k^T, v (stream m in chunks of 512)
            for mc in range(M // 512):
                ctx_tiles = []
                for s in range(4):
                    t = ld_pool.tile([128, DC], F32, tag="ld")
                    nc.sync.dma_start(
                        out=t, in_=ctx_d[(mc * 4 + s) * 128:(mc * 4 + s + 1) * 128, :])
                    ctx_tiles.append(t)
                ctxT = tT_pool.tile([128, FC, 512], F32R, tag="tT")
                for fi in range(FC):
                    ps = tp_psum.tile([128, 512], F32, tag="tp")
                    for s in range(4):
                        nc.tensor.transpose(
                            ps[:, s * 128:(s + 1) * 128],
                            ctx_tiles[s][:, fi * 128:(fi + 1) * 128], ident)
                    nc.vector.tensor_copy(out=ctxT[:, fi, :], in_=ps)
                for p2 in range(2):  # k^T d-tiles
                    ps = mm_psum.tile([128, 512], F32, tag="mm")
                    for fi in range(FC):
                        nc.tensor.matmul(
                            ps,
                            wk_sb[:, fi, p2 * 128:(p2 + 1) * 128],
                            ctxT[:, fi, :],
                            start=(fi == 0), stop=(fi == FC - 1))
                    nc.vector.tensor_copy(
                        out=kT_sb[:, p2, mc * 512:(mc + 1) * 512], in_=ps)
                for s in range(4):  # v m-subtiles
                    mt = mc * 4 + s
                    ps = v_psum.tile([128, GD], F32, tag="v")
                    for fi in range(FC):
                        nc.tensor.matmul(
                            ps,
                            ctxT[:, fi, s * 128:(s + 1) * 128],
                            wv_sb[:, fi, :],
                            start=(fi == 0), stop=(fi == FC - 1))
                    for h in range(HPC):
                        nc.vector.tensor_copy(
                            out=v_sb[:, mt, h * VW : h * VW + DH],
                            in_=ps[:, h * DH:(h + 1) * DH])

            # x -> q^T (stream n in chunks of 512)
            for ncK in range(N // 512):
                x_tiles = []
                for s in range(4):
                    t = ld_pool.tile([128, DQ], F32, tag="ld")
                    nc.sync.dma_start(
                        out=t, in_=x_d[(ncK * 4 + s) * 128:(ncK * 4 + s + 1) * 128, :])
                    x_tiles.append(t)
                xT = tT_pool.tile([128, FQ, 512], F32R, tag="tT")
                for fi in range(FQ):
                    ps = tp_psum.tile([128, 512], F32, tag="tp")
                    for s in range(4):
                        nc.tensor.transpose(
                            ps[:, s * 128:(s + 1) * 128],
                            x_tiles[s][:, fi * 128:(fi + 1) * 128], ident)
                    nc.vector.tensor_copy(out=xT[:, fi, :], in_=ps)
                for p2 in range(2):
                    ps = mm_psum.tile([128, 512], F32, tag="mm")
                    for fi in range(FQ):
                        nc.tensor.matmul(
                            ps,
                            wq_sb[:, fi, p2 * 128:(p2 + 1) * 128],
                            xT[:, fi, :],
                            start=(fi == 0), stop=(fi == FQ - 1))
                    nc.vector.tensor_copy(
                        out=qT_sb[:, p2, ncK * 512:(ncK + 1) * 512], in_=ps)

        # ---------------- Phase B: attention ----------------
        NC2 = 1024  # n-chunk for the score/exp/attnV pipeline
        with (
            tc.tile_pool(name="st_ps", bufs=2, space="PSUM") as st_psum,
            tc.tile_pool(name="acc_ps", bufs=2, space="PSUM") as acc_psum,
            tc.tile_pool(name="pT", bufs=3) as p_pool,
            tc.tile_pool(name="div", bufs=2) as div_pool,
        ):
            for h in range(HPC):
                pair, ro = divmod(h, 2)
                ro *= DH
                kTh = kT_sb[ro:ro + DH, pair, :]
                qTh = qT_sb[ro:ro + DH, pair, :]
                for ncK in range(N // NC2):
                    acc = acc_psum.tile([VW, NC2], F32, tag="acc")
                    for mt in range(MT):
                        st = st_psum.tile([128, NC2], F32, tag="st")
                        for hf in range(NC2 // 512):
                            nc.tensor.matmul(
                                st[:, hf * 512:(hf + 1) * 512],
                                kTh[:, mt * 128:(mt + 1) * 128],
                                qTh[:, ncK * NC2 + hf * 512:
                                    ncK * NC2 + (hf + 1) * 512],
                                start=True, stop=True)
                        pT = p_pool.tile([128, NC2], F32R, tag="pT")
                        nc.scalar.activation(
                            out=pT, in_=st, func=AF.Exp,
                            bias=lm_sb[:, mt:mt + 1], scale=SCALE)
                        for hf in range(NC2 // 512):
                            nc.tensor.matmul(
                                acc[:, hf * 512:(hf + 1) * 512],
                                v_sb[:, mt, h * VW:(h + 1) * VW],
                                pT[:, hf * 512:(hf + 1) * 512],
                                start=(mt == 0), stop=(mt == MT - 1))
                    # normalize: o^T_h = acc[0:64] / acc[64]
                    rec = div_pool.tile([1, NC2], F32, tag="rec")
                    nc.vector.reciprocal(out=rec, in_=acc[DH:DH + 1, :])
                    bc = div_pool.tile([DH, NC2], F32, tag="bc")
                    nc.gpsimd.partition_broadcast(bc, rec)
                    nc.vector.tensor_mul(
                        out=oT_sb[ro:ro + DH, pair, ncK * NC2:(ncK + 1) * NC2],
                        in0=acc[0:DH, :], in1=bc)

        # ---------------- Phase C: output projection ----------------
        with (
            tc.tile_pool(name="fin_ps", bufs=2, space="PSUM") as fin_psum,
            tc.tile_pool(name="fin_sb", bufs=2) as fin_pool,
        ):
            for nt in range(NT):
                ps = fin_psum.tile([128, DQ], F32, tag="fin")
                for hf in range(DQ // 512):
                    for pair in range(2):
                        nc.tensor.matmul(
                            ps[:, hf * 512:(hf + 1) * 512],
                            oT_sb[:, pair, nt * 128:(nt + 1) * 128],
                            wo_sb[:, pair, hf * 512:(hf + 1) * 512],
                            start=(pair == 0), stop=(pair == 1))
                fs = fin_pool.tile([128, DQ], F32, tag="fs")
                nc.vector.tensor_copy(out=fs, in_=ps)
                nc.sync.dma_start(out=out_d[nt * 128:(nt + 1) * 128, :], in_=fs)


@lru_cache(maxsize=1)
def build_program():
    nc = bacc.Bacc("TRN2", target_bir_lowering=False, debug=False,
                   num_devices=N_CORES)
    ins = {
        "x": nc.dram_tensor("x", [N, DQ], F32, kind="ExternalInput").ap(),
        "ctx": nc.dram_tensor("ctx", [M, DC], F32, kind="ExternalInput").ap(),
        "wq": nc.dram_tensor("wq", [DQ, GD], F32R, kind="ExternalInput").ap(),
        "wk": nc.dram_tensor("wk", [DC, GD], F32R, kind="ExternalInput").ap(),
        "wv": nc.dram_tensor("wv", [DC, GD], F32R, kind="ExternalInput").ap(),
        "wo": nc.dram_tensor("wo", [GD, DQ], F32R, kind="ExternalInput").ap(),
        "lm": nc.dram_tensor("lm", [128, MT], F32, kind="ExternalInput").ap(),
    }
    outs = {
        "out": nc.dram_tensor("out", [N, DQ], F32, kind="ExternalOutput").ap(),
    }
    with tile.TileContext(nc) as tc:
        build_kernel(tc, ins, outs)
    nc.compile()
    return nc


def make_in_maps(x, context, context_mask, Wq, Wk, Wv, Wo):
    in_maps = []
    for c in range(N_CORES):
        b, g = divmod(c, HPC)
        gs = slice(g * GD, (g + 1) * GD)
        lm = np.where(context_mask[b], 0.0, NEG).astype(np.float32)
        in_maps.append({
            "x": np.ascontiguousarray(x[b], np.float32),
            "ctx": np.ascontiguousarray(context[b], np.float32),
            "wq": np.ascontiguousarray(Wq[:, gs], np.float32),
            "wk": np.ascontiguousarray(Wk[:, gs], np.float32),
            "wv": np.ascontiguousarray(Wv[:, gs], np.float32),
            "wo": np.ascontiguousarray(Wo[gs, :], np.float32),
            "lm": np.ascontiguousarray(lm.reshape(MT, 128).T),
        })
    return in_maps


def assemble_output(results, bo):
    out = np.zeros((B, N, DQ), np.float32)
    for c in range(N_CORES):
        out[c // HPC] += results[c]["out"]
    out += np.asarray(bo, np.float32)
    return out


def kernel(**inputs):
    x = np.asarray(inputs["x"], np.float32)
    context = np.asarray(inputs["context"], np.float32)
    mask = np.asarray(inputs["context_mask"])
    nc = build_program()
    in_maps = make_in_maps(x, context, mask,
                           inputs["Wq"], inputs["Wk"], inputs["Wv"], inputs["Wo"])
    res = run_bass_kernel_spmd(nc, in_maps, core_ids=list(range(N_CORES)))
    return assemble_output(res.results, inputs["bo"])


if __name__ == "__main__":
    rng = np.random.default_rng(0)
    ins = {
        "x": rng.normal(size=(B, N, DQ)).astype(np.float32),
        "context": rng.normal(size=(B, M, DC)).astype(np.float32),
        "context_mask": np.ones((B, M), bool),
        "Wq": (rng.normal(size=(DQ, H * DH)) * 0.02).astype(np.float32),
        "Wk": (rng.normal(size=(DC, H * DH)) * 0.02).astype(np.float32),
        "Wv": (rng.normal(size=(DC, H * DH)) * 0.02).astype(np.float32),
        "Wo": (rng.normal(size=(H * DH, DQ)) * 0.02).astype(np.float32),
        "bo": np.zeros((DQ,), np.float32),
    }
    out = kernel(**ins)
    print("out", out.shape, out.dtype, float(np.abs(out).mean()))
